# revision 38
# baseline (speedup 1.0000x reference)
"""AttentionPooling (segment softmax pooling) Trainium2 Bass kernel.

Math (equivalent to the reference, max-subtraction dropped -- gate logits are
~N(0,1) so exp() is safe in fp32, and the softmax is shift-invariant):
    g = x @ Wg + bg                      per node
    e = exp(g)
    u[s]     = sum_{i in s} e_i * (x_i @ Wm)       (bias folded out)
    denom[s] = sum_{i in s} e_i
    out[s]   = u[s] / (denom[s] + EPS) + bm        (gates sum to 1 per segment)

Sharding: 4096 segments -> 8 cores x 4 blocks of 128 segments. index is
sorted, so each (core, block) owns a contiguous row range; rows are padded to
a uniform tile count TB so every core runs the same SPMD program.

Per 128-node tile on device:
  - msg+gate GEMM: lhsT = x^T chunks (stationary), rhs = [Wg | Wm | pad]
    split as N=256 + N=258 fp32r matmuls (N>=256 -> full PE rate; fp32r dst
    needs even widths), accumulated over 4 K-chunks into two PSUM banks.
    Column 0 is the gate logit.
  - e = exp(g + bg) on ScalarE.
  - E[i,j] = (rel_i == j) * e_i built on VectorE (one tensor_scalar, chained
    is_equal * mult); rel is the host-computed segment id relative to the
    tile's 128-segment block; pad rows get rel = -1 -> zero column.
  - pooling: u += E^T @ w and denom += E^T @ ones accumulate in PSUM across
    all tiles of the block (w = e * m, split over ScalarE/VectorE).
Block flush: rdenom = 1/(denom+EPS) on DVE, out = u * rdenom + bm, DMA out.
"""

import os
import sys

for _p in ("/opt/trn_rl_repo", "/root/.axon_site/_ro/trn_rl_repo"):
    if os.path.isdir(_p) and _p not in sys.path:
        sys.path.insert(0, _p)

import numpy as np

N_NODES = 262144
D = 512
S = 4096
N_CORES = 8
SEGS_PER_CORE = S // N_CORES          # 512
BLOCKS = SEGS_PER_CORE // 128         # 4
EPS = 1e-10

last_results = None  # BassKernelResults of the most recent run (for test.py)


def _build(nc, TB, cfg=None):
    import concourse.mybir as mybir
    from concourse.tile import TileContext
    from contextlib import ExitStack

    cfg = cfg or {}
    WCOPY_MOD = cfg.get("wcopy_mod", 3)   # every Nth tile's w-copy on DVE (0=all ACT)
    WORK_BUFS = cfg.get("work_bufs", 3)
    MA_BUFS = cfg.get("ma_bufs", 3)
    X_BUFS = cfg.get("x_bufs", 2)

    f32 = mybir.dt.float32
    f32r = mybir.dt.float32r
    AF = mybir.ActivationFunctionType
    Alu = mybir.AluOpType

    NT = BLOCKS * TB          # tiles per core
    NPAD = NT * 128           # padded rows per core

    # matmul operands are float32r (fp32 rounded to 11-bit mantissa); host
    # pre-rounds xt/wp so the DMA'd bits are already valid f32r
    xt_d = nc.dram_tensor("xt", [4, 128, NPAD], f32r, kind="ExternalInput")
    relT_d = nc.dram_tensor("relT", [128, NT], f32, kind="ExternalInput")
    # [Wg | Wm | 0-pad] -> 514 cols so both matmul splits have even free dims
    # (fp32r matmul dst requires even column counts)
    wp_d = nc.dram_tensor("wp", [D, D + 2], f32r, kind="ExternalInput")
    bgb_d = nc.dram_tensor("bgb", [128, 1], f32, kind="ExternalInput")
    bmb_d = nc.dram_tensor("bmb", [128, D], f32, kind="ExternalInput")
    iota_d = nc.dram_tensor("iota", [128, 128], f32, kind="ExternalInput")
    ones_d = nc.dram_tensor("ones", [128, 2], f32r, kind="ExternalInput")
    out_d = nc.dram_tensor("out", [SEGS_PER_CORE, D], f32, kind="ExternalOutput")

    with TileContext(nc) as tc, ExitStack() as ctx:
        const = ctx.enter_context(tc.tile_pool(name="const", bufs=1))
        xpool = ctx.enter_context(tc.tile_pool(name="xp", bufs=X_BUFS))
        psmA = ctx.enter_context(
            tc.tile_pool(name="psmA", bufs=MA_BUFS, space="PSUM")
        )
        psmB = ctx.enter_context(tc.tile_pool(name="psmB", bufs=2, space="PSUM"))
        psu = ctx.enter_context(tc.tile_pool(name="psu", bufs=2, space="PSUM"))
        psd = ctx.enter_context(tc.tile_pool(name="psd", bufs=1, space="PSUM"))
        work = ctx.enter_context(tc.tile_pool(name="wk", bufs=WORK_BUFS))
        opool = ctx.enter_context(tc.tile_pool(name="op", bufs=2))

        # resident constants
        wp_sb = []
        for c in range(4):
            t = const.tile([128, D + 2], f32r, tag=f"wp{c}", name=f"wp{c}")
            nc.sync.dma_start(t[:], wp_d[c * 128:(c + 1) * 128, :])
            wp_sb.append(t)
        iota_sb = const.tile([128, 128], f32, tag="iota")
        nc.sync.dma_start(iota_sb[:], iota_d[:, :])
        relT_sb = const.tile([128, NT], f32, tag="relT")
        nc.sync.dma_start(relT_sb[:], relT_d[:, :])
        bgb_sb = const.tile([128, 1], f32, tag="bgb")
        nc.sync.dma_start(bgb_sb[:], bgb_d[:, :])
        bmb_sb = const.tile([128, D], f32, tag="bmb")
        nc.sync.dma_start(bmb_sb[:], bmb_d[:, :])
        ones_sb = const.tile([128, 2], f32r, tag="ones")
        nc.sync.dma_start(ones_sb[:], ones_d[:, :])

        denom_ps = psd.tile([128, 2 * BLOCKS], f32, tag="denom")

        # PE warmup: dummy matmuls on the (early-DMA'd) weight tiles while the
        # first x tiles are still in flight, so the HAM clock gate is released
        # before real work starts. Results go to a scratch bank, never read.
        n_warm = cfg.get("warmup", 12)
        if n_warm:
            # scratch shares the u pool slot; block 0's first pool matmul
            # uses start=True so the garbage is cleared before accumulation
            scratch_ps = psu.tile([128, 512], f32, tag="u", name="scratch")
            for i in range(n_warm):
                nc.tensor.matmul(
                    scratch_ps[:], wp_sb[i % 4][:, 0:128],
                    wp_sb[i % 4][:, 0:512],
                    start=True, stop=True, skip_group_check=True,
                )

        xts = None
        for j in range(BLOCKS):
            u_ps = psu.tile([128, D], f32, tag="u")
            for t in range(TB):
                gt = j * TB + t
                g4 = gt % 4
                if g4 == 0:
                    xts = [
                        xpool.tile([128, 512], f32r, tag=f"x{c}", name=f"x{c}")
                        for c in range(4)
                    ]
                    for c in range(4):
                        nc.sync.dma_start(
                            xts[c][:], xt_d[c, :, gt * 128: gt * 128 + 512]
                        )
                mA = psmA.tile([128, 256], f32, tag="mA")
                mB = psmB.tile([128, 258], f32, tag="mB")
                for c in range(4):
                    lhsT = xts[c][:, g4 * 128:(g4 + 1) * 128]
                    nc.tensor.matmul(
                        mA[:], lhsT, wp_sb[c][:, 0:256],
                        start=(c == 0), stop=(c == 3), skip_group_check=True,
                    )
                    nc.tensor.matmul(
                        mB[:], lhsT, wp_sb[c][:, 256:514],
                        start=(c == 0), stop=(c == 3), skip_group_check=True,
                    )
                e = work.tile([128, 1], f32, tag="e")
                nc.scalar.activation(e[:], mA[:, 0:1], AF.Exp, bias=bgb_sb[:])
                E = work.tile([128, 128], f32r, tag="E")
                nc.vector.tensor_scalar(
                    E[:], iota_sb[:], relT_sb[:, gt:gt + 1], e[:],
                    op0=Alu.is_equal, op1=Alu.mult,
                )
                # plain PSUM->SBUF copy of the message rows; the gate weight e
                # is already folded into E, so no scaling here
                w = work.tile([128, D], f32r, tag="w")
                if WCOPY_MOD and gt % WCOPY_MOD == 0:
                    nc.vector.tensor_copy(w[:, 0:255], mA[:, 1:256])
                    nc.vector.tensor_copy(w[:, 255:512], mB[:, 0:257])
                else:
                    nc.scalar.activation(w[:, 0:255], mA[:, 1:256], AF.Copy)
                    nc.scalar.activation(w[:, 255:512], mB[:, 0:257], AF.Copy)
                nc.tensor.matmul(
                    u_ps[:], E[:], w[:],
                    start=(t == 0), stop=(t == TB - 1), skip_group_check=True,
                )
                nc.tensor.matmul(
                    denom_ps[:, 2 * j:2 * j + 2], E[:], ones_sb[:],
                    start=(t == 0), stop=(t == TB - 1), skip_group_check=True,
                )
            rd = work.tile([128, 1], f32, tag="rd")
            nc.vector.tensor_scalar(
                rd[:], denom_ps[:, 2 * j:2 * j + 1], EPS, None, op0=Alu.add
            )
            nc.vector.reciprocal(rd[:], rd[:])
            ob = opool.tile([128, D], f32, tag="ob")
            nc.scalar.activation(ob[:], u_ps[:], AF.Identity, scale=rd[:])
            nc.vector.tensor_add(ob[:], ob[:], bmb_sb[:])
            nc.sync.dma_start(out_d[j * 128:(j + 1) * 128, :], ob[:])
    return nc


def _round_f32r(a):
    """Round fp32 -> fp32r (11-bit mantissa, RNE; low 12 bits zero)."""
    b = np.ascontiguousarray(a, dtype=np.float32).view(np.uint32)
    r = (b + np.uint32(0x7FF) + ((b >> np.uint32(12)) & np.uint32(1))) & np.uint32(
        0xFFFFF000
    )
    return r.view(np.float32)


def kernel(x, index, Wg, bg, Wm, bm, num_segments):
    global last_results
    import concourse.bacc as bacc
    from concourse.bass_utils import run_bass_kernel_spmd

    x = np.ascontiguousarray(np.asarray(x, dtype=np.float32))
    index = np.asarray(index).astype(np.int64)
    Wg = np.asarray(Wg, dtype=np.float32).reshape(D, 1)
    bg = np.asarray(bg, dtype=np.float32).reshape(1)
    Wm = np.asarray(Wm, dtype=np.float32).reshape(D, D)
    bm = np.asarray(bm, dtype=np.float32).reshape(D)
    assert int(num_segments) == S and x.shape == (N_NODES, D)

    counts = np.bincount(index, minlength=S).astype(np.int64)
    seg_off = np.zeros(S + 1, dtype=np.int64)
    np.cumsum(counts, out=seg_off[1:])
    blk_rows = counts.reshape(N_CORES, BLOCKS, 128).sum(-1)  # [8, 4]
    TB = int(np.ceil(blk_rows.max() / 128))
    NT = BLOCKS * TB  # always divisible by 4 (BLOCKS=4) for grouped x loads
    NPAD = NT * 128

    Wp = _round_f32r(
        np.concatenate([Wg, Wm, np.zeros((D, 1), np.float32)], axis=1)
    )  # [512, 514] = [Wg | Wm | pad]
    x = _round_f32r(x)
    iota = np.ascontiguousarray(
        np.tile(np.arange(128, dtype=np.float32)[None, :], (128, 1))
    )
    bgb = np.full((128, 1), float(bg[0]), dtype=np.float32)
    bmb = np.ascontiguousarray(np.tile(bm[None, :], (128, 1)))

    in_maps = []
    for c in range(N_CORES):
        xt = np.zeros((4, 128, NPAD), dtype=np.float32)
        relT = np.full((128, NT), -1.0, dtype=np.float32)
        for j in range(BLOCKS):
            s0 = seg_off[c * SEGS_PER_CORE + j * 128]
            s1 = seg_off[c * SEGS_PER_CORE + (j + 1) * 128]
            r = int(s1 - s0)
            off = j * TB * 128
            xt[:, :, off:off + r] = x[s0:s1].T.reshape(4, 128, r)
            relpad = np.full(TB * 128, -1.0, dtype=np.float32)
            relpad[:r] = (index[s0:s1] - (c * SEGS_PER_CORE + j * 128)).astype(
                np.float32
            )
            relT[:, j * TB:(j + 1) * TB] = relpad.reshape(TB, 128).T
        in_maps.append(
            {"xt": xt, "relT": relT, "wp": Wp, "bgb": bgb, "bmb": bmb,
             "iota": iota, "ones": np.ones((128, 2), dtype=np.float32)}
        )

    nc = bacc.Bacc(trn_type="TRN2")
    _build(nc, TB)
    nc.compile()
    last_results = run_bass_kernel_spmd(
        nc, in_maps, core_ids=list(range(N_CORES))
    )
    out = np.concatenate(
        [r["out"] for r in last_results.results], axis=0
    ).astype(np.float32)
    return out


# revision 39
# speedup vs baseline: 1.0017x; 1.0017x over previous
"""AttentionPooling (segment softmax pooling) Trainium2 Bass kernel.

Math (equivalent to the reference, max-subtraction dropped -- gate logits are
~N(0,1) so exp() is safe in fp32, and the softmax is shift-invariant):
    g = x @ Wg + bg                      per node
    e = exp(g)
    u[s]     = sum_{i in s} e_i * (x_i @ Wm)       (bias folded out)
    denom[s] = sum_{i in s} e_i
    out[s]   = u[s] / (denom[s] + EPS) + bm        (gates sum to 1 per segment)

Sharding: 4096 segments -> 8 cores x 4 blocks of 128 segments. index is
sorted, so each (core, block) owns a contiguous row range; rows are padded to
a uniform tile count TB so every core runs the same SPMD program.

Per 128-node tile on device:
  - msg+gate GEMM: lhsT = x^T chunks (stationary), rhs = [Wg | Wm | pad]
    split as N=256 + N=258 fp32r matmuls (N>=256 -> full PE rate; fp32r dst
    needs even widths), accumulated over 4 K-chunks into two PSUM banks.
    Column 0 is the gate logit.
  - e = exp(g + bg) on ScalarE.
  - E[i,j] = (rel_i == j) * e_i built on VectorE (one tensor_scalar, chained
    is_equal * mult); rel is the host-computed segment id relative to the
    tile's 128-segment block; pad rows get rel = -1 -> zero column.
  - pooling: u += E^T @ w and denom += E^T @ ones accumulate in PSUM across
    all tiles of the block (w = e * m, split over ScalarE/VectorE).
Block flush: rdenom = 1/(denom+EPS) on DVE, out = u * rdenom + bm, DMA out.
"""

import os
import sys

for _p in ("/opt/trn_rl_repo", "/root/.axon_site/_ro/trn_rl_repo"):
    if os.path.isdir(_p) and _p not in sys.path:
        sys.path.insert(0, _p)

import numpy as np

N_NODES = 262144
D = 512
S = 4096
N_CORES = 8
SEGS_PER_CORE = S // N_CORES          # 512
BLOCKS = SEGS_PER_CORE // 128         # 4
EPS = 1e-10

last_results = None  # BassKernelResults of the most recent run (for test.py)


def _build(nc, TB, cfg=None):
    import concourse.mybir as mybir
    from concourse.tile import TileContext
    from contextlib import ExitStack

    cfg = cfg or {}
    WCOPY_MOD = cfg.get("wcopy_mod", 3)   # every Nth tile's w-copy on DVE (0=all ACT)
    WORK_BUFS = cfg.get("work_bufs", 3)
    MA_BUFS = cfg.get("ma_bufs", 3)
    X_BUFS = cfg.get("x_bufs", 2)

    f32 = mybir.dt.float32
    f32r = mybir.dt.float32r
    AF = mybir.ActivationFunctionType
    Alu = mybir.AluOpType

    NT = BLOCKS * TB          # tiles per core
    NPAD = NT * 128           # padded rows per core

    # matmul operands are float32r (fp32 rounded to 11-bit mantissa); host
    # pre-rounds xt/wp so the DMA'd bits are already valid f32r
    xt_d = nc.dram_tensor("xt", [4, 128, NPAD], f32r, kind="ExternalInput")
    relT_d = nc.dram_tensor("relT", [128, NT], f32, kind="ExternalInput")
    # [Wg | Wm | 0-pad] -> 514 cols so both matmul splits have even free dims
    # (fp32r matmul dst requires even column counts)
    wp_d = nc.dram_tensor("wp", [D, D + 2], f32r, kind="ExternalInput")
    bgb_d = nc.dram_tensor("bgb", [128, 1], f32, kind="ExternalInput")
    bmb_d = nc.dram_tensor("bmb", [128, D], f32, kind="ExternalInput")
    iota_d = nc.dram_tensor("iota", [128, 128], f32, kind="ExternalInput")
    ones_d = nc.dram_tensor("ones", [128, 2], f32r, kind="ExternalInput")
    out_d = nc.dram_tensor("out", [SEGS_PER_CORE, D], f32, kind="ExternalOutput")

    with TileContext(nc) as tc, ExitStack() as ctx:
        const = ctx.enter_context(tc.tile_pool(name="const", bufs=1))
        xpool = ctx.enter_context(tc.tile_pool(name="xp", bufs=X_BUFS))
        psmA = ctx.enter_context(
            tc.tile_pool(name="psmA", bufs=MA_BUFS, space="PSUM")
        )
        psmB = ctx.enter_context(tc.tile_pool(name="psmB", bufs=2, space="PSUM"))
        psu = ctx.enter_context(tc.tile_pool(name="psu", bufs=2, space="PSUM"))
        psd = ctx.enter_context(tc.tile_pool(name="psd", bufs=1, space="PSUM"))
        work = ctx.enter_context(tc.tile_pool(name="wk", bufs=WORK_BUFS))
        opool = ctx.enter_context(tc.tile_pool(name="op", bufs=2))

        # resident constants
        wp_sb = []
        for c in range(4):
            t = const.tile([128, D + 2], f32r, tag=f"wp{c}", name=f"wp{c}")
            nc.sync.dma_start(t[:], wp_d[c * 128:(c + 1) * 128, :])
            wp_sb.append(t)
        iota_sb = const.tile([128, 128], f32, tag="iota")
        nc.sync.dma_start(iota_sb[:], iota_d[:, :])
        relT_sb = const.tile([128, NT], f32, tag="relT")
        nc.sync.dma_start(relT_sb[:], relT_d[:, :])
        bgb_sb = const.tile([128, 1], f32, tag="bgb")
        nc.sync.dma_start(bgb_sb[:], bgb_d[:, :])
        bmb_sb = const.tile([128, D], f32, tag="bmb")
        nc.sync.dma_start(bmb_sb[:], bmb_d[:, :])
        ones_sb = const.tile([128, 2], f32r, tag="ones")
        nc.sync.dma_start(ones_sb[:], ones_d[:, :])

        denom_ps = psd.tile([128, 2 * BLOCKS], f32, tag="denom")

        # PE warmup: dummy matmuls on the (early-DMA'd) weight tiles while the
        # first x tiles are still in flight, so the HAM clock gate is released
        # before real work starts. Results go to a scratch bank, never read.
        n_warm = cfg.get("warmup", 10)
        if n_warm:
            # scratch shares the u pool slot; block 0's first pool matmul
            # uses start=True so the garbage is cleared before accumulation
            scratch_ps = psu.tile([128, 512], f32, tag="u", name="scratch")
            for i in range(n_warm):
                nc.tensor.matmul(
                    scratch_ps[:], wp_sb[i % 4][:, 0:128],
                    wp_sb[i % 4][:, 0:512],
                    start=True, stop=True, skip_group_check=True,
                )

        xts = None
        for j in range(BLOCKS):
            u_ps = psu.tile([128, D], f32, tag="u")
            for t in range(TB):
                gt = j * TB + t
                g4 = gt % 4
                if g4 == 0:
                    xts = [
                        xpool.tile([128, 512], f32r, tag=f"x{c}", name=f"x{c}")
                        for c in range(4)
                    ]
                    for c in range(4):
                        nc.sync.dma_start(
                            xts[c][:], xt_d[c, :, gt * 128: gt * 128 + 512]
                        )
                mA = psmA.tile([128, 256], f32, tag="mA")
                mB = psmB.tile([128, 258], f32, tag="mB")
                for c in range(4):
                    lhsT = xts[c][:, g4 * 128:(g4 + 1) * 128]
                    nc.tensor.matmul(
                        mA[:], lhsT, wp_sb[c][:, 0:256],
                        start=(c == 0), stop=(c == 3), skip_group_check=True,
                    )
                    nc.tensor.matmul(
                        mB[:], lhsT, wp_sb[c][:, 256:514],
                        start=(c == 0), stop=(c == 3), skip_group_check=True,
                    )
                e = work.tile([128, 1], f32, tag="e")
                nc.scalar.activation(e[:], mA[:, 0:1], AF.Exp, bias=bgb_sb[:])
                E = work.tile([128, 128], f32r, tag="E")
                nc.vector.tensor_scalar(
                    E[:], iota_sb[:], relT_sb[:, gt:gt + 1], e[:],
                    op0=Alu.is_equal, op1=Alu.mult,
                )
                # plain PSUM->SBUF copy of the message rows; the gate weight e
                # is already folded into E, so no scaling here
                w = work.tile([128, D], f32r, tag="w")
                if WCOPY_MOD and gt % WCOPY_MOD == 0:
                    nc.vector.tensor_copy(w[:, 0:255], mA[:, 1:256])
                    nc.vector.tensor_copy(w[:, 255:512], mB[:, 0:257])
                else:
                    nc.scalar.activation(w[:, 0:255], mA[:, 1:256], AF.Copy)
                    nc.scalar.activation(w[:, 255:512], mB[:, 0:257], AF.Copy)
                nc.tensor.matmul(
                    u_ps[:], E[:], w[:],
                    start=(t == 0), stop=(t == TB - 1), skip_group_check=True,
                )
                nc.tensor.matmul(
                    denom_ps[:, 2 * j:2 * j + 2], E[:], ones_sb[:],
                    start=(t == 0), stop=(t == TB - 1), skip_group_check=True,
                )
            rd = work.tile([128, 1], f32, tag="rd")
            nc.vector.tensor_scalar(
                rd[:], denom_ps[:, 2 * j:2 * j + 1], EPS, None, op0=Alu.add
            )
            nc.vector.reciprocal(rd[:], rd[:])
            ob = opool.tile([128, D], f32, tag="ob")
            nc.scalar.activation(ob[:], u_ps[:], AF.Identity, scale=rd[:])
            nc.vector.tensor_add(ob[:], ob[:], bmb_sb[:])
            nc.sync.dma_start(out_d[j * 128:(j + 1) * 128, :], ob[:])
    return nc


def _round_f32r(a):
    """Round fp32 -> fp32r (11-bit mantissa, RNE; low 12 bits zero)."""
    b = np.ascontiguousarray(a, dtype=np.float32).view(np.uint32)
    r = (b + np.uint32(0x7FF) + ((b >> np.uint32(12)) & np.uint32(1))) & np.uint32(
        0xFFFFF000
    )
    return r.view(np.float32)


def kernel(x, index, Wg, bg, Wm, bm, num_segments):
    global last_results
    import concourse.bacc as bacc
    from concourse.bass_utils import run_bass_kernel_spmd

    x = np.ascontiguousarray(np.asarray(x, dtype=np.float32))
    index = np.asarray(index).astype(np.int64)
    Wg = np.asarray(Wg, dtype=np.float32).reshape(D, 1)
    bg = np.asarray(bg, dtype=np.float32).reshape(1)
    Wm = np.asarray(Wm, dtype=np.float32).reshape(D, D)
    bm = np.asarray(bm, dtype=np.float32).reshape(D)
    assert int(num_segments) == S and x.shape == (N_NODES, D)

    counts = np.bincount(index, minlength=S).astype(np.int64)
    seg_off = np.zeros(S + 1, dtype=np.int64)
    np.cumsum(counts, out=seg_off[1:])
    blk_rows = counts.reshape(N_CORES, BLOCKS, 128).sum(-1)  # [8, 4]
    TB = int(np.ceil(blk_rows.max() / 128))
    NT = BLOCKS * TB  # always divisible by 4 (BLOCKS=4) for grouped x loads
    NPAD = NT * 128

    Wp = _round_f32r(
        np.concatenate([Wg, Wm, np.zeros((D, 1), np.float32)], axis=1)
    )  # [512, 514] = [Wg | Wm | pad]
    x = _round_f32r(x)
    iota = np.ascontiguousarray(
        np.tile(np.arange(128, dtype=np.float32)[None, :], (128, 1))
    )
    bgb = np.full((128, 1), float(bg[0]), dtype=np.float32)
    bmb = np.ascontiguousarray(np.tile(bm[None, :], (128, 1)))

    in_maps = []
    for c in range(N_CORES):
        xt = np.zeros((4, 128, NPAD), dtype=np.float32)
        relT = np.full((128, NT), -1.0, dtype=np.float32)
        for j in range(BLOCKS):
            s0 = seg_off[c * SEGS_PER_CORE + j * 128]
            s1 = seg_off[c * SEGS_PER_CORE + (j + 1) * 128]
            r = int(s1 - s0)
            off = j * TB * 128
            xt[:, :, off:off + r] = x[s0:s1].T.reshape(4, 128, r)
            relpad = np.full(TB * 128, -1.0, dtype=np.float32)
            relpad[:r] = (index[s0:s1] - (c * SEGS_PER_CORE + j * 128)).astype(
                np.float32
            )
            relT[:, j * TB:(j + 1) * TB] = relpad.reshape(TB, 128).T
        in_maps.append(
            {"xt": xt, "relT": relT, "wp": Wp, "bgb": bgb, "bmb": bmb,
             "iota": iota, "ones": np.ones((128, 2), dtype=np.float32)}
        )

    nc = bacc.Bacc(trn_type="TRN2")
    _build(nc, TB)
    nc.compile()
    last_results = run_bass_kernel_spmd(
        nc, in_maps, core_ids=list(range(N_CORES))
    )
    out = np.concatenate(
        [r["out"] for r in last_results.results], axis=0
    ).astype(np.float32)
    return out
